# revision 34
# baseline (speedup 1.0000x reference)
"""Trainium2 Bass kernel for nn_BalancedInputNeurons (moe_routing).

Reference computation (B=4, S=1024, H=768, N=64, R=16):
    act   = sigmoid(l2norm(x) @ l2norm(patterns).T)            (B,S,N)
    shared = gelu_exact(x @ W_shared + b_shared)               (B,S,H)
    down  = einsum('bsh,nhr->bsnr', shared, adapt_down)
    spec  = einsum('bsnr,bsn,nrh->bsh', down, act, adapt_up)
    out   = shared * act.sum(-1, keepdims=True) + spec
    returns (out, act)

Strategy: data-parallel over the 4096 tokens across 8 cores (512 tokens
each), replicating all parameters.  On-chip compute is feature-major
(H or N*R on partitions, tokens on the free axis) so every matmul
contraction sits on the partition axis; the gating path runs token-major
so the per-token 1/||x|| scale and sigmoid can use per-partition
activation scaling.  The per-(n,token) activation gate is broadcast to
(n*r,token) partition groups with a selection-matrix matmul on the PE.
Adapters travel as fp8(e4m3) scaled by 256; the 2^-16 on the selection
matrix undoes both scales exactly (powers of two), so only the fp8
quantization itself (~3.6% on the tiny `spec` correction) remains.
"""

import numpy as np
import ml_dtypes

import concourse.bass as bass
import concourse.mybir as mybir
import concourse.tile as tile
from concourse.vector_clock import ScopedClock
from concourse.masks import make_identity
from concourse.bass import ts
from concourse.bass_utils import run_bass_kernel_spmd

B, S, H, N, R = 4, 1024, 768, 64, 16
NCORES = 8
TOK = B * S              # 4096 tokens total
T = TOK // NCORES        # 512 tokens per core
KH = H // 128            # 6 k-tiles over H
NR = N * R               # 1024
KNR = NR // 128          # 8 k-tiles over N*R
TI = T // 128            # 4 token tiles per core
GRP = 128 // R           # 8 neurons per 128-partition group
WARM_MMS = 8

f32 = mybir.dt.float32
bf16 = mybir.dt.bfloat16
fp8 = mybir.dt.float8e4
AF = mybir.ActivationFunctionType


_MAX_WAITS = 1  # this walrus build rejects >1 sync-wait per instruction
_nop_counter = [0]


def _split_excess_waits(nc: bass.Bass):
    """Walrus here accepts only one sync-wait command per instruction.
    Hoist extra waits onto injected same-engine NoOps placed just before
    the over-subscribed instruction (engine program order = block order,
    so the waits still complete before the instruction issues)."""
    for f in nc.m.functions:
        for blk in f.blocks:
            insts = blk.instructions
            out = []
            for inst in insts:
                si = inst.sync_info
                if si is not None and si.on_wait is not None and len(si.on_wait) > _MAX_WAITS:
                    waits = list(si.on_wait)
                    extra, keep = waits[:-_MAX_WAITS], waits[-_MAX_WAITS:]
                    for wchunk in extra:
                        _nop_counter[0] += 1
                        nop = mybir.InstNoOp(
                            name=f"I-waitnop-{_nop_counter[0]}", ins=[], outs=[]
                        )
                        nop.engine = inst.engine
                        nop.sync_info = mybir.SyncInfo(
                            on_wait=[wchunk], on_update=[]
                        )
                        out.append(nop)
                    si.on_wait = keep
                out.append(inst)
            blk.instructions = out


class _SplitDrainTileContext(tile.TileContext):
    def _drain_and_barrier(self, tick_clock, wait_clock):
        drain_inst = self.nc.sync.drain()
        wait_clock.add_sem_waits(
            drain_inst.ins, ScopedClock({None: tick_clock.global_clock})
        )
        self.nc.all_engine_barrier()
        assert self.sems is not None
        popped = self.nc._tile_sem_poison_stack.pop()
        assert popped is self._sem_poison
        self.nc.clear_and_free_semaphores(list(self.sems.allocated().values()))
        self.nc.all_engine_barrier()
        _split_excess_waits(self.nc)


def build_nc(loop_n: int = 1) -> bass.Bass:
    """Build the per-core Bass program.  loop_n > 1 wraps the whole body in
    a hardware For_i loop (for timing measurements via slope)."""
    nc = bass.Bass()

    xT = nc.dram_tensor("xT", [H, T], bf16, kind="ExternalInput")
    xtm = nc.dram_tensor("xtm", [T, H], bf16, kind="ExternalInput")
    pnT = nc.dram_tensor("pnT", [H, N], bf16, kind="ExternalInput")
    w = nc.dram_tensor("w", [H, H], bf16, kind="ExternalInput")
    bsh = nc.dram_tensor("bsh", [128, KH], f32, kind="ExternalInput")
    ad = nc.dram_tensor("ad", [H, NR], fp8, kind="ExternalInput")
    au = nc.dram_tensor("au", [NR, H], fp8, kind="ExternalInput")
    esel = nc.dram_tensor("esel", [N, NR], bf16, kind="ExternalInput")
    outT = nc.dram_tensor("outT", [H, T], f32, kind="ExternalOutput")
    act = nc.dram_tensor("act", [T, N], f32, kind="ExternalOutput")

    with _SplitDrainTileContext(nc) as tc:
        with (
            tc.tile_pool(name="consts", bufs=1) as consts,
            tc.tile_pool(name="weights", bufs=1) as wpool,
            tc.tile_pool(name="acts", bufs=1) as apool,
            tc.tile_pool(name="work", bufs=2) as work,
            tc.tile_pool(name="small", bufs=8) as small,
            tc.tile_pool(name="mm", bufs=7, space="PSUM") as mmp,
            tc.tile_pool(name="persist", bufs=1, space="PSUM") as pp,
        ):
            ident = consts.tile([128, 128], f32)
            make_identity(nc, ident)
            ones64 = consts.tile([64, 128], bf16)
            nc.vector.memset(ones64, 1.0)
            warm = consts.tile([128, 128], bf16)
            nc.vector.memset(warm, 1.0)

            # one DMA per tensor: the HWDGE pipe costs ~625ns fixed per
            # dma_start, so instruction count dominates transfer size
            xT_sb = [wpool.tile([128, KH // 2, T], bf16, name=f"xTh{h}") for h in range(2)]
            xtm_sb = wpool.tile([128, TI, H], bf16)
            pn_sb = wpool.tile([128, KH, N], bf16)
            w_sb = [wpool.tile([128, KH // 2, H], bf16, name=f"wh{h}") for h in range(2)]
            b_sb = wpool.tile([128, KH], f32)
            ad_sb = wpool.tile([128, KH, NR], fp8)
            au_sb = wpool.tile([128, KNR, H], fp8)
            ab_sb = [wpool.tile([128, T], bf16, name=f"abg{g}") for g in range(KNR)]
            esel_sb = wpool.tile([64, NR], bf16)

            def xT_k(k):
                return xT_sb[k // (KH // 2)][:, k % (KH // 2), :]

            def w_k(k):
                return w_sb[k // (KH // 2)][:, k % (KH // 2), :]

            def body(_iv=None):
                # ---- input DMAs: gating deps first; big weights go on the
                # Activation-engine HWDGE queue so the two HW queues split
                # the ~7 MB of traffic ----
                xT_r = xT[:].rearrange("(k p) t -> p k t", p=128)
                xtm_r = xtm[:].rearrange("(i p) h -> p i h", p=128)
                pn_r = pnT[:].rearrange("(k p) n -> p k n", p=128)
                w_r = w[:].rearrange("(k p) o -> p k o", p=128)
                ad_r = ad[:].rearrange("(k p) q -> p k q", p=128)
                au_r = au[:].rearrange("(k p) h -> p k h", p=128)
                nc.sync.dma_start(pn_sb[:], pn_r)
                nc.sync.dma_start(xT_sb[0][:], xT_r[:, 0 : KH // 2, :])
                nc.sync.dma_start(xT_sb[1][:], xT_r[:, KH // 2 :, :])
                nc.sync.dma_start(w_sb[0][:], w_r[:, 0 : KH // 2, :])
                nc.sync.dma_start(xtm_sb[:], xtm_r)
                nc.sync.dma_start(w_sb[1][:], w_r[:, KH // 2 :, :])
                nc.sync.dma_start(b_sb[:], bsh[:])
                nc.sync.dma_start(ad_sb[:], ad_r)
                nc.sync.dma_start(esel_sb[:], esel[:])
                nc.sync.dma_start(au_sb[:], au_r)

                shared_f = [apool.tile([128, T], f32, name=f"shf{m}") for m in range(KH)]
                shared_b = [apool.tile([128, T], bf16, name=f"shb{m}") for m in range(KH)]
                smix = [apool.tile([128, T], f32, name=f"smx{m}") for m in range(KH)]
                dsc_sb = [apool.tile([128, T], bf16, name=f"dsc{g}") for g in range(KNR)]
                actfm_ps = pp.tile([64, T], f32, space="PSUM")
                actsum_ps = mmp.tile([128, T], f32, tag="mm", space="PSUM")

                dummy = small.tile([128, 1], f32, tag="dmy")
                nc.vector.memset(dummy[:], 0.0)
                nc.scalar.activation(out=dummy[:], in_=dummy[:], func=AF.Sigmoid)

                # PE warm-up: keep the PE busy while the first DMAs land so
                # the HAM clock-gate ramp completes before real matmuls
                warm_ps = mmp.tile([128, 128], f32, tag="mm", space="PSUM")
                for _ in range(WARM_MMS):
                    nc.tensor.matmul(
                        warm_ps[:], lhsT=warm[:], rhs=warm[:],
                        start=True, stop=True,
                    )

                # ---- phase 1: per-token 1/||x|| on DVE (no ACT tables) ----
                # invn = rsqrt(normsq) via bit-trick + 2 Newton steps, per
                # token-tile so sigmoid i=0 unblocks as early as possible.
                normsq4 = small.tile([128, TI], f32, tag="nrm")
                invn4 = small.tile([128, TI], f32, tag="nrm")
                nwt = small.tile([128, TI], f32, tag="nrm")
                nwt2 = small.tile([128, TI], f32, tag="nrm")
                for i in range(TI):
                    sq = work.tile([128, H], bf16, tag="sq")
                    ns_i = normsq4[:, i : i + 1]
                    iv_i = invn4[:, i : i + 1]
                    n1_i = nwt[:, i : i + 1]
                    n2_i = nwt2[:, i : i + 1]
                    nc.scalar.activation(
                        out=sq[:], in_=xtm_sb[:, i, :], func=AF.Square,
                        accum_out=ns_i,
                    )
                    nc.vector.tensor_scalar(
                        out=iv_i.bitcast(mybir.dt.int32),
                        in0=ns_i.bitcast(mybir.dt.int32),
                        scalar1=1, scalar2=None,
                        op0=mybir.AluOpType.logical_shift_right,
                    )
                    nc.vector.tensor_scalar(
                        out=iv_i.bitcast(mybir.dt.int32),
                        in0=iv_i.bitcast(mybir.dt.int32),
                        scalar1=0x5F3759DF, scalar2=-1,
                        op0=mybir.AluOpType.subtract,
                        op1=mybir.AluOpType.mult,
                    )
                    for _ in range(2):
                        nc.vector.tensor_mul(n1_i, ns_i, iv_i)
                        nc.vector.tensor_mul(n2_i, n1_i, iv_i)
                        nc.vector.tensor_scalar(
                            out=n2_i, in0=n2_i,
                            scalar1=-0.5, scalar2=1.5,
                            op0=mybir.AluOpType.mult, op1=mybir.AluOpType.add,
                        )
                        nc.vector.tensor_mul(iv_i, iv_i, n2_i)

                # ---- phase 1b: gating matmuls + sigmoid (token-major) ----
                act_all = apool.tile([128, TI, N], f32)
                for i in range(TI):
                    raw = mmp.tile([128, N], f32, tag="mm", space="PSUM")
                    for k in range(KH):
                        nc.tensor.matmul(
                            raw[:],
                            lhsT=xT_k(k)[:, ts(i, 128)],
                            rhs=pn_sb[:, k, :],
                            start=(k == 0),
                            stop=(k == KH - 1),
                        )
                    raw_sb = work.tile([128, N], f32, tag="rawsb", bufs=4)
                    nc.vector.tensor_copy(raw_sb[:], raw[:])
                    nc.scalar.activation(
                        out=act_all[:, i, :], in_=raw_sb[:], func=AF.Sigmoid,
                        scale=invn4[:, i : i + 1],
                    )
                nc.sync.dma_start(
                    act[:].rearrange("(i p) n -> p i n", p=128), act_all[:]
                )

                # ---- phase 2: shared = gelu(x @ W + b), feature-major ----
                for m in range(KH):
                    sh_ps = mmp.tile([128, T], f32, tag="mm", space="PSUM")
                    for k in range(KH):
                        nc.tensor.matmul(
                            sh_ps[:],
                            lhsT=w_k(k)[:, ts(m, 128)],
                            rhs=xT_k(k),
                            start=(k == 0),
                            stop=(k == KH - 1),
                        )
                    nc.scalar.activation(
                        out=shared_f[m][:], in_=sh_ps[:], func=AF.Gelu,
                        bias=b_sb[:, m : m + 1], scale=1.0,
                    )
                    nc.vector.tensor_copy(shared_b[m][:], shared_f[m][:])

                # ---- phase 2b: act transpose, broadcast (PE), act-sum ----
                for i in range(TI):
                    nc.tensor.transpose(
                        actfm_ps[:, ts(i, 128)], act_all[:, i, :], ident[:]
                    )
                actfm_sb = apool.tile([64, T], bf16)
                nc.vector.tensor_copy(actfm_sb[:], actfm_ps[:])
                for g in range(KNR):
                    ab_ps = mmp.tile([128, T], f32, tag="mm", space="PSUM")
                    nc.tensor.matmul(
                        ab_ps[:], lhsT=esel_sb[:, ts(g, 128)], rhs=actfm_sb[:],
                        start=True, stop=True,
                    )
                    nc.scalar.copy(ab_sb[g][:], ab_ps[:])
                nc.tensor.matmul(
                    actsum_ps[:], lhsT=ones64[:], rhs=actfm_sb[:],
                    start=True, stop=True,
                )
                actsum_sb = apool.tile([128, T], f32)
                nc.vector.tensor_copy(actsum_sb[:], actsum_ps[:])
                # act_sum premix on GpSimd (frees DVE for the gate/combine)
                for m in range(KH):
                    nc.gpsimd.tensor_mul(
                        smix[m][:], shared_f[m][:], actsum_sb[:]
                    )

                # ---- phase 3: down projection + gate, feature-major ----
                for g in range(KNR):
                    dn_ps = mmp.tile([128, T], f32, tag="mm", space="PSUM")
                    for k in range(KH):
                        nc.tensor.matmul(
                            dn_ps[:],
                            lhsT=ad_sb[:, k, ts(g, 128)],
                            rhs=shared_b[k][:],
                            start=(k == 0),
                            stop=(k == KH - 1),
                        )
                    nc.vector.tensor_mul(dsc_sb[g][:], dn_ps[:], ab_sb[g][:])

                # ---- phase 4: up projection + combine ----
                for m in range(KH):
                    sp_ps = mmp.tile([128, T], f32, tag="mm", space="PSUM")
                    for k in range(KNR):
                        nc.tensor.matmul(
                            sp_ps[:],
                            lhsT=au_sb[:, k, ts(m, 128)],
                            rhs=dsc_sb[k][:],
                            start=(k == 0),
                            stop=(k == KNR - 1),
                        )
                    out_m = work.tile([128, T], f32, tag="outm")
                    for hh in range(2):
                        sl = slice(hh * (T // 2), (hh + 1) * (T // 2))
                        nc.vector.tensor_add(
                            out_m[:, sl], smix[m][:, sl], sp_ps[:, sl]
                        )
                        nc.sync.dma_start(outT[ts(m, 128), sl], out_m[:, sl])

            if loop_n > 1:
                with tc.For_i(
                    0, loop_n, 1,
                    staggered_reset=True,
                    hint_engines=(
                        mybir.EngineType.PE,
                        mybir.EngineType.SP,
                        mybir.EngineType.DVE,
                        mybir.EngineType.Activation,
                        mybir.EngineType.Pool,
                    ),
                ) as iv:
                    body(iv)
            else:
                body()

    return nc


_NC_CACHE: dict = {}


def _get_nc(loop_n: int = 1) -> bass.Bass:
    if loop_n not in _NC_CACHE:
        _NC_CACHE[loop_n] = build_nc(loop_n)
    return _NC_CACHE[loop_n]


def make_in_maps(x, patterns, W_shared, b_shared, adapt_down, adapt_up):
    """Host-side sharding + layout prep (numpy only)."""
    x = np.asarray(x, dtype=np.float32)
    patterns = np.asarray(patterns, dtype=np.float32)
    W_shared = np.asarray(W_shared, dtype=np.float32)
    b_shared = np.asarray(b_shared, dtype=np.float32)
    adapt_down = np.asarray(adapt_down, dtype=np.float32)
    adapt_up = np.asarray(adapt_up, dtype=np.float32)

    bft = ml_dtypes.bfloat16
    x_flat = x.reshape(TOK, H)
    pn = patterns / np.maximum(
        np.linalg.norm(patterns, axis=-1, keepdims=True), 1e-12
    )
    pnT_h = np.ascontiguousarray(pn.T).astype(bft)                 # (H, N)
    w_h = np.ascontiguousarray(W_shared).astype(bft)               # (H, H)
    b_h = np.ascontiguousarray(b_shared.reshape(KH, 128).T).astype(np.float32)
    # adapters in fp8 e4m3, scaled by 256 (power of two => exact folding);
    # the 2^-16 on the selection matrix undoes both scales exactly.
    f8 = ml_dtypes.float8_e4m3
    ad_h = np.ascontiguousarray(
        adapt_down.transpose(1, 0, 2).reshape(H, NR) * 256.0
    ).astype(f8)                                                   # (H, N*R)
    au_h = np.ascontiguousarray(adapt_up.reshape(NR, H) * 256.0).astype(f8)
    esel_h = (
        (np.arange(N)[:, None] == (np.arange(NR)[None, :] // R)) * 2.0**-16
    ).astype(bft)

    in_maps = []
    for c in range(NCORES):
        xs = x_flat[c * T : (c + 1) * T]                           # (T, H)
        in_maps.append(
            {
                "xT": np.ascontiguousarray(xs.T).astype(bft),
                "xtm": np.ascontiguousarray(xs).astype(bft),
                "pnT": pnT_h,
                "w": w_h,
                "bsh": b_h,
                "ad": ad_h,
                "au": au_h,
                "esel": esel_h,
            }
        )
    return in_maps


def assemble(results):
    inter = np.concatenate(
        [np.asarray(r["outT"], dtype=np.float32).T for r in results], axis=0
    ).reshape(B, S, H)
    acts = np.concatenate(
        [np.asarray(r["act"], dtype=np.float32) for r in results], axis=0
    ).reshape(B, S, N)
    return inter, acts


def kernel(x, patterns, W_shared, b_shared, adapt_down, adapt_up):
    nc = _get_nc(1)
    in_maps = make_in_maps(x, patterns, W_shared, b_shared, adapt_down, adapt_up)
    res = run_bass_kernel_spmd(nc, in_maps, core_ids=list(range(NCORES)))
    return assemble(res.results)


# revision 43
# speedup vs baseline: 1.2968x; 1.2968x over previous
"""Trainium2 Bass kernel for nn_BalancedInputNeurons (moe_routing).

Reference computation (B=4, S=1024, H=768, N=64, R=16):
    act   = sigmoid(l2norm(x) @ l2norm(patterns).T)            (B,S,N)
    shared = gelu_exact(x @ W_shared + b_shared)               (B,S,H)
    down  = einsum('bsh,nhr->bsnr', shared, adapt_down)
    spec  = einsum('bsnr,bsn,nrh->bsh', down, act, adapt_up)
    out   = shared * act.sum(-1, keepdims=True) + spec
    returns (out, act)

Strategy: data-parallel over the 4096 tokens across 8 cores (512 tokens
each), replicating all parameters.  On-chip compute is feature-major
(H or N*R on partitions, tokens on the free axis) so every matmul
contraction sits on the partition axis; the gating path runs token-major
so the per-token 1/||x|| scale and sigmoid can use per-partition
activation scaling.  The per-(n,token) activation gate is broadcast to
(n*r,token) partition groups via a DRAM round trip with partition-
broadcast access patterns (step-0 AP dims), scaled by 2^-16 on the way.
Adapters travel as fp8(e4m3) scaled by 256 each; 256*256*2^-16 = 1, all
powers of two, so only the fp8 quantization itself (~3.6% on the tiny
`spec` correction) remains.
"""

import numpy as np
import ml_dtypes

import concourse.bass as bass
import concourse.mybir as mybir
import concourse.tile as tile
from concourse.vector_clock import ScopedClock
from concourse.masks import make_identity
from concourse.bass import ts
from concourse.bass_utils import run_bass_kernel_spmd

B, S, H, N, R = 4, 1024, 768, 64, 16
NCORES = 8
TOK = B * S              # 4096 tokens total
T = TOK // NCORES        # 512 tokens per core
KH = H // 128            # 6 k-tiles over H
NR = N * R               # 1024
KNR = NR // 128          # 8 k-tiles over N*R
TI = T // 128            # 4 token tiles per core
GRP = 128 // R           # 8 neurons per 128-partition group
WARM_MMS = 8

f32 = mybir.dt.float32
bf16 = mybir.dt.bfloat16
fp8 = mybir.dt.float8e4
AF = mybir.ActivationFunctionType


_MAX_WAITS = 1  # this walrus build rejects >1 sync-wait per instruction
_nop_counter = [0]


def _split_excess_waits(nc: bass.Bass):
    """Walrus here accepts only one sync-wait command per instruction.
    Hoist extra waits onto injected same-engine NoOps placed just before
    the over-subscribed instruction (engine program order = block order,
    so the waits still complete before the instruction issues)."""
    for f in nc.m.functions:
        for blk in f.blocks:
            insts = blk.instructions
            out = []
            for inst in insts:
                si = inst.sync_info
                if si is not None and si.on_wait is not None and len(si.on_wait) > _MAX_WAITS:
                    waits = list(si.on_wait)
                    extra, keep = waits[:-_MAX_WAITS], waits[-_MAX_WAITS:]
                    for wchunk in extra:
                        _nop_counter[0] += 1
                        nop = mybir.InstNoOp(
                            name=f"I-waitnop-{_nop_counter[0]}", ins=[], outs=[]
                        )
                        nop.engine = inst.engine
                        nop.sync_info = mybir.SyncInfo(
                            on_wait=[wchunk], on_update=[]
                        )
                        out.append(nop)
                    si.on_wait = keep
                out.append(inst)
            blk.instructions = out


class _SplitDrainTileContext(tile.TileContext):
    def _drain_and_barrier(self, tick_clock, wait_clock):
        drain_inst = self.nc.sync.drain()
        wait_clock.add_sem_waits(
            drain_inst.ins, ScopedClock({None: tick_clock.global_clock})
        )
        self.nc.all_engine_barrier()
        assert self.sems is not None
        popped = self.nc._tile_sem_poison_stack.pop()
        assert popped is self._sem_poison
        self.nc.clear_and_free_semaphores(list(self.sems.allocated().values()))
        _split_excess_waits(self.nc)


def build_nc(loop_n: int = 1) -> bass.Bass:
    """Build the per-core Bass program.  loop_n > 1 wraps the whole body in
    a hardware For_i loop (for timing measurements via slope)."""
    nc = bass.Bass()

    xT = nc.dram_tensor("xT", [H, T], bf16, kind="ExternalInput")
    xtm = nc.dram_tensor("xtm", [T, H], bf16, kind="ExternalInput")
    pnT = nc.dram_tensor("pnT", [H, N], bf16, kind="ExternalInput")
    w = nc.dram_tensor("w", [H, H], bf16, kind="ExternalInput")
    bsh = nc.dram_tensor("bsh", [128, KH], f32, kind="ExternalInput")
    ad = nc.dram_tensor("ad", [H, NR], fp8, kind="ExternalInput")
    au = nc.dram_tensor("au", [NR, H], fp8, kind="ExternalInput")
    outT = nc.dram_tensor("outT", [H, T], f32, kind="ExternalOutput")
    act = nc.dram_tensor("act", [T, N], f32, kind="ExternalOutput")

    with _SplitDrainTileContext(nc) as tc:
        with (
            tc.tile_pool(name="consts", bufs=1) as consts,
            tc.tile_pool(name="weights", bufs=1) as wpool,
            tc.tile_pool(name="acts", bufs=1) as apool,
            tc.tile_pool(name="work", bufs=2) as work,
            tc.tile_pool(name="small", bufs=8) as small,
            tc.tile_pool(name="mm", bufs=7, space="PSUM") as mmp,
            tc.tile_pool(name="persist", bufs=1, space="PSUM") as pp,
            tc.tile_pool(name="dram", bufs=1, space="DRAM") as dpool,
        ):
            ident = consts.tile([128, 128], f32)
            make_identity(nc, ident)
            ones64 = consts.tile([64, 128], bf16)
            nc.vector.memset(ones64, 1.0)
            warm = consts.tile([128, 128], bf16)
            nc.gpsimd.memset(warm, 1.0)

            # one DMA per tensor: the HWDGE pipe costs ~625ns fixed per
            # dma_start, so instruction count dominates transfer size
            xT_sb = [wpool.tile([128, KH // 2, T], bf16, name=f"xTh{h}") for h in range(2)]
            xtm_sb = wpool.tile([128, TI, H], bf16)
            pn_sb = wpool.tile([128, KH, N], bf16)
            w_sb = [wpool.tile([128, KH // 2, H], bf16, name=f"wh{h}") for h in range(2)]
            b_sb = wpool.tile([128, KH], f32)
            ad_sb = wpool.tile([128, KH, NR], fp8)
            au_sb = wpool.tile([128, KNR, H], fp8)
            ab_sb = [wpool.tile([128, T], bf16, name=f"abg{g}") for g in range(KNR)]

            def xT_k(k):
                return xT_sb[k // (KH // 2)][:, k % (KH // 2), :]

            def w_k(k):
                return w_sb[k // (KH // 2)][:, k % (KH // 2), :]

            def body(_iv=None):
                # ---- input DMAs: gating deps first; big weights go on the
                # Activation-engine HWDGE queue so the two HW queues split
                # the ~7 MB of traffic ----
                xT_r = xT[:].rearrange("(k p) t -> p k t", p=128)
                xtm_r = xtm[:].rearrange("(i p) h -> p i h", p=128)
                pn_r = pnT[:].rearrange("(k p) n -> p k n", p=128)
                w_r = w[:].rearrange("(k p) o -> p k o", p=128)
                ad_r = ad[:].rearrange("(k p) q -> p k q", p=128)
                au_r = au[:].rearrange("(k p) h -> p k h", p=128)
                nc.sync.dma_start(pn_sb[:], pn_r)
                nc.sync.dma_start(xT_sb[0][:], xT_r[:, 0 : KH // 2, :])
                nc.sync.dma_start(xT_sb[1][:], xT_r[:, KH // 2 :, :])
                nc.sync.dma_start(w_sb[0][:], w_r[:, 0 : KH // 2, :])
                nc.sync.dma_start(xtm_sb[:], xtm_r)
                nc.sync.dma_start(w_sb[1][:], w_r[:, KH // 2 :, :])
                nc.sync.dma_start(b_sb[:], bsh[:])
                nc.sync.dma_start(ad_sb[:], ad_r)
                nc.sync.dma_start(au_sb[:], au_r)

                shared_f = [apool.tile([128, T], f32, name=f"shf{m}") for m in range(KH)]
                shared_b = [apool.tile([128, T], bf16, name=f"shb{m}") for m in range(KH)]
                smix = [apool.tile([128, T], f32, name=f"smx{m}") for m in range(KH)]
                dsc_sb = [apool.tile([128, T], bf16, name=f"dsc{g}") for g in range(KNR)]
                actfm_ps = pp.tile([64, T], f32, space="PSUM")
                actsum_ps = mmp.tile([128, T], f32, tag="mm", space="PSUM")

                dummy = small.tile([128, 1], f32, tag="dmy")
                nc.vector.memset(dummy[:], 0.0)
                nc.scalar.activation(out=dummy[:], in_=dummy[:], func=AF.Sigmoid)

                # PE warm-up: keep the PE busy while the first DMAs land so
                # the HAM clock-gate ramp completes before real matmuls
                warm_ps = mmp.tile([128, 128], f32, tag="mm", space="PSUM")
                for _ in range(WARM_MMS):
                    nc.tensor.matmul(
                        warm_ps[:], lhsT=warm[:], rhs=warm[:],
                        start=True, stop=True,
                    )

                # ---- phase 1: per-token 1/||x|| on DVE (no ACT tables) ----
                # invn = rsqrt(normsq) via bit-trick + 2 Newton steps, per
                # token-tile so sigmoid i=0 unblocks as early as possible.
                normsq4 = small.tile([128, TI], f32, tag="nrm")
                invn4 = small.tile([128, TI], f32, tag="nrm")
                nwt = small.tile([128, TI], f32, tag="nrm")
                nwt2 = small.tile([128, TI], f32, tag="nrm")
                for i in range(TI):
                    sq = work.tile([128, H], bf16, tag="sq")
                    ns_i = normsq4[:, i : i + 1]
                    iv_i = invn4[:, i : i + 1]
                    n1_i = nwt[:, i : i + 1]
                    n2_i = nwt2[:, i : i + 1]
                    nc.scalar.activation(
                        out=sq[:], in_=xtm_sb[:, i, :], func=AF.Square,
                        accum_out=ns_i,
                    )
                    nc.vector.tensor_scalar(
                        out=iv_i.bitcast(mybir.dt.int32),
                        in0=ns_i.bitcast(mybir.dt.int32),
                        scalar1=1, scalar2=None,
                        op0=mybir.AluOpType.logical_shift_right,
                    )
                    nc.vector.tensor_scalar(
                        out=iv_i.bitcast(mybir.dt.int32),
                        in0=iv_i.bitcast(mybir.dt.int32),
                        scalar1=0x5F3759DF, scalar2=-1,
                        op0=mybir.AluOpType.subtract,
                        op1=mybir.AluOpType.mult,
                    )
                    for _ in range(2):
                        nc.vector.tensor_mul(n1_i, ns_i, iv_i)
                        nc.vector.tensor_mul(n2_i, n1_i, iv_i)
                        nc.vector.tensor_scalar(
                            out=n2_i, in0=n2_i,
                            scalar1=-0.5, scalar2=1.5,
                            op0=mybir.AluOpType.mult, op1=mybir.AluOpType.add,
                        )
                        nc.vector.tensor_mul(iv_i, iv_i, n2_i)

                # ---- phase 1b: gating matmuls + sigmoid (token-major) ----
                act_all = apool.tile([128, TI, N], f32)
                for i in range(TI):
                    raw = mmp.tile([128, N], f32, tag="mm", space="PSUM")
                    for k in range(KH):
                        nc.tensor.matmul(
                            raw[:],
                            lhsT=xT_k(k)[:, ts(i, 128)],
                            rhs=pn_sb[:, k, :],
                            start=(k == 0),
                            stop=(k == KH - 1),
                        )
                    raw_sb = work.tile([128, N], f32, tag="rawsb", bufs=4)
                    nc.vector.tensor_copy(raw_sb[:], raw[:])
                    nc.scalar.activation(
                        out=act_all[:, i, :], in_=raw_sb[:], func=AF.Sigmoid,
                        scale=invn4[:, i : i + 1],
                    )
                nc.sync.dma_start(
                    act[:].rearrange("(i p) n -> p i n", p=128), act_all[:]
                )

                # ---- phase 2: shared = gelu(x @ W + b), feature-major ----
                for m in range(KH):
                    sh_ps = mmp.tile([128, T], f32, tag="mm", space="PSUM")
                    for k in range(KH):
                        nc.tensor.matmul(
                            sh_ps[:],
                            lhsT=w_k(k)[:, ts(m, 128)],
                            rhs=xT_k(k),
                            start=(k == 0),
                            stop=(k == KH - 1),
                        )
                    nc.scalar.activation(
                        out=shared_f[m][:], in_=sh_ps[:], func=AF.Gelu,
                        bias=b_sb[:, m : m + 1], scale=1.0,
                    )
                    nc.vector.tensor_copy(shared_b[m][:], shared_f[m][:])

                # ---- phase 2b: act transpose, broadcast (PE), act-sum ----
                for i in range(TI):
                    nc.tensor.transpose(
                        actfm_ps[:, ts(i, 128)], act_all[:, i, :], ident[:]
                    )
                actfm_sb = apool.tile([64, T], bf16)
                nc.vector.tensor_copy(actfm_sb[:], actfm_ps[:])
                actfm_sc = apool.tile([64, T], bf16)
                nc.vector.tensor_scalar_mul(actfm_sc[:], in0=actfm_ps[:], scalar1=2.0**-16)
                actfm_dram = dpool.tile([N, T], bf16, space="DRAM")
                nc.sync.dma_start(actfm_dram[:], actfm_sc[:])
                for g in range(KNR):
                    sgrp = actfm_dram[g * GRP : (g + 1) * GRP, :]
                    bcast = bass.AP(
                        tensor=sgrp.tensor, offset=sgrp.offset,
                        ap=[sgrp.ap[0], [0, R], sgrp.ap[1]],
                    )
                    nc.sync.dma_start(ab_sb[g][:], bcast)
                nc.tensor.matmul(
                    actsum_ps[:], lhsT=ones64[:], rhs=actfm_sb[:],
                    start=True, stop=True,
                )
                actsum_sb = apool.tile([128, T], f32)
                nc.vector.tensor_copy(actsum_sb[:], actsum_ps[:])
                # act_sum premix on GpSimd (frees DVE for the gate/combine)
                for m in range(KH):
                    nc.gpsimd.tensor_mul(
                        smix[m][:], shared_f[m][:], actsum_sb[:]
                    )

                # ---- phase 3: down projection + gate, feature-major ----
                for g in range(KNR):
                    dn_ps = mmp.tile([128, T], f32, tag="mm", space="PSUM")
                    for k in range(KH):
                        nc.tensor.matmul(
                            dn_ps[:],
                            lhsT=ad_sb[:, k, ts(g, 128)],
                            rhs=shared_b[k][:],
                            start=(k == 0),
                            stop=(k == KH - 1),
                        )
                    nc.vector.tensor_mul(dsc_sb[g][:], dn_ps[:], ab_sb[g][:])

                # ---- phase 4: up projection + combine ----
                for m in range(KH):
                    sp_ps = mmp.tile([128, T], f32, tag="mm", space="PSUM")
                    for k in range(KNR):
                        nc.tensor.matmul(
                            sp_ps[:],
                            lhsT=au_sb[:, k, ts(m, 128)],
                            rhs=dsc_sb[k][:],
                            start=(k == 0),
                            stop=(k == KNR - 1),
                        )
                    out_m = work.tile([128, T], f32, tag="outm")
                    for hh in range(2):
                        sl = slice(hh * (T // 2), (hh + 1) * (T // 2))
                        nc.vector.tensor_add(
                            out_m[:, sl], smix[m][:, sl], sp_ps[:, sl]
                        )
                        nc.sync.dma_start(outT[ts(m, 128), sl], out_m[:, sl])

            if loop_n > 1:
                with tc.For_i(
                    0, loop_n, 1,
                    staggered_reset=True,
                    hint_engines=(
                        mybir.EngineType.PE,
                        mybir.EngineType.SP,
                        mybir.EngineType.DVE,
                        mybir.EngineType.Activation,
                        mybir.EngineType.Pool,
                    ),
                ) as iv:
                    body(iv)
            else:
                body()

    return nc


_NC_CACHE: dict = {}


def _get_nc(loop_n: int = 1) -> bass.Bass:
    if loop_n not in _NC_CACHE:
        _NC_CACHE[loop_n] = build_nc(loop_n)
    return _NC_CACHE[loop_n]


def make_in_maps(x, patterns, W_shared, b_shared, adapt_down, adapt_up):
    """Host-side sharding + layout prep (numpy only)."""
    x = np.asarray(x, dtype=np.float32)
    patterns = np.asarray(patterns, dtype=np.float32)
    W_shared = np.asarray(W_shared, dtype=np.float32)
    b_shared = np.asarray(b_shared, dtype=np.float32)
    adapt_down = np.asarray(adapt_down, dtype=np.float32)
    adapt_up = np.asarray(adapt_up, dtype=np.float32)

    bft = ml_dtypes.bfloat16
    x_flat = x.reshape(TOK, H)
    pn = patterns / np.maximum(
        np.linalg.norm(patterns, axis=-1, keepdims=True), 1e-12
    )
    pnT_h = np.ascontiguousarray(pn.T).astype(bft)                 # (H, N)
    w_h = np.ascontiguousarray(W_shared).astype(bft)               # (H, H)
    b_h = np.ascontiguousarray(b_shared.reshape(KH, 128).T).astype(np.float32)
    # adapters in fp8 e4m3, scaled by 256 (power of two => exact folding);
    # the 2^-16 on the selection matrix undoes both scales exactly.
    f8 = ml_dtypes.float8_e4m3
    ad_h = np.ascontiguousarray(
        adapt_down.transpose(1, 0, 2).reshape(H, NR) * 256.0
    ).astype(f8)                                                   # (H, N*R)
    au_h = np.ascontiguousarray(adapt_up.reshape(NR, H) * 256.0).astype(f8)

    in_maps = []
    for c in range(NCORES):
        xs = x_flat[c * T : (c + 1) * T]                           # (T, H)
        in_maps.append(
            {
                "xT": np.ascontiguousarray(xs.T).astype(bft),
                "xtm": np.ascontiguousarray(xs).astype(bft),
                "pnT": pnT_h,
                "w": w_h,
                "bsh": b_h,
                "ad": ad_h,
                "au": au_h,
            }
        )
    return in_maps


def assemble(results):
    inter = np.concatenate(
        [np.asarray(r["outT"], dtype=np.float32).T for r in results], axis=0
    ).reshape(B, S, H)
    acts = np.concatenate(
        [np.asarray(r["act"], dtype=np.float32) for r in results], axis=0
    ).reshape(B, S, N)
    return inter, acts


def kernel(x, patterns, W_shared, b_shared, adapt_down, adapt_up):
    nc = _get_nc(1)
    in_maps = make_in_maps(x, patterns, W_shared, b_shared, adapt_down, adapt_up)
    res = run_bass_kernel_spmd(nc, in_maps, core_ids=list(range(NCORES)))
    return assemble(res.results)
